# revision 1
# baseline (speedup 1.0000x reference)
"""Trainium2 Bass kernel for HGCN adaptive hyperedge top-k masking.

Computes, for fixed shapes B=8, N=10000, M=4000, K=20:
    adj = relu(tanh(2 * tanh(2*nodevec) @ tanh(2*edgevec).T))   [N, M]
    top-20 columns per row of (adj + 0.01*noise), emitted as a global
    hyperedge list HE [2, B*N*K] (int32) with weights HEW [B*N*K] (f32).

Sharding: rows (N) split across 8 NeuronCores, 1250 rows each; nodevec
shard + edgevec replicated on-chip; batch dim is a pure repeat done on host.

Device per 128-row tile: fp32 matmul (contraction 40) -> PSUM, ACT tanh,
DVE scalar_tensor_tensor (relu + noise add), then 3 rounds of
max8/max_index/match_replace to extract the top-24 (values, indices).
Host: sort indices, recover adj weights, assemble HE/HEW.
"""

import sys

if "/opt/trn_rl_repo" not in sys.path:
    sys.path.insert(0, "/opt/trn_rl_repo")

import numpy as np

B, N, M, K = 8, 10000, 4000, 20
NCORES = 8
RPC = N // NCORES          # rows per core: 1250
P = 128                    # partitions
R = 24                     # extracted per row (3 rounds of max8)
D = 40                     # embedding dim
FREE = 512                 # psum bank width (f32)
NCHUNK = (M + FREE - 1) // FREE   # 8 column chunks (last = 416)
NTILES = (RPC + P - 1) // P       # 10 row tiles per core (last = 98)

_compiled = {}


def _build():
    import concourse.bacc as bacc
    import concourse.mybir as mybir
    from concourse.tile import TileContext

    dt = mybir.dt
    Alu = mybir.AluOpType
    Act = mybir.ActivationFunctionType

    nc = bacc.Bacc("TRN2", debug=False, num_devices=NCORES)

    nvT = nc.dram_tensor("nvT", [D, RPC], dt.float32, kind="ExternalInput")
    evT = nc.dram_tensor("evT", [D, M], dt.float32, kind="ExternalInput")
    ns = nc.dram_tensor("ns", [RPC, M], dt.float32, kind="ExternalInput")
    ov = nc.dram_tensor("ov", [RPC, R], dt.float32, kind="ExternalOutput")
    oi = nc.dram_tensor("oi", [RPC, R], dt.uint32, kind="ExternalOutput")

    with TileContext(nc) as tc:
        with (
            tc.tile_pool(name="embed", bufs=1) as epool,
            tc.tile_pool(name="noise", bufs=3) as npool,
            tc.tile_pool(name="work", bufs=2) as wpool,
            tc.tile_pool(name="ps", bufs=1, space="PSUM") as pspool,
            tc.tile_pool(name="outs", bufs=3) as opool,
        ):
            det = epool.tile([D, RPC], dt.float32, tag="det")
            eet = epool.tile([D, M], dt.float32, tag="eet")
            nc.sync.dma_start(det[:, :], nvT[:, :])
            nc.sync.dma_start(eet[:, :], evT[:, :])
            nc.scalar.activation(det[:, :], det[:, :], Act.Tanh, scale=2.0)
            nc.scalar.activation(eet[:, :], eet[:, :], Act.Tanh, scale=2.0)

            for t in range(NTILES):
                rows = min(P, RPC - t * P)
                r0 = t * P

                tt = wpool.tile([P, M], dt.float32, tag="tanh")
                pts = []
                for j in range(NCHUNK):
                    w = min(FREE, M - j * FREE)
                    pt = pspool.tile([P, FREE], dt.float32, tag=f"ps{j}")
                    nc.tensor.matmul(
                        pt[:rows, :w],
                        det[:, r0 : r0 + rows],
                        eet[:, j * FREE : j * FREE + w],
                        start=True,
                        stop=True,
                    )
                    pts.append((pt, w))
                for j, (pt, w) in enumerate(pts):
                    nc.scalar.activation(
                        tt[:rows, j * FREE : j * FREE + w],
                        pt[:rows, :w],
                        Act.Tanh,
                        scale=2.0,
                    )

                nst = npool.tile([P, M], dt.float32, tag="ns")
                nc.sync.dma_start(nst[:rows, :], ns[r0 : r0 + rows, :])

                # v = max(tanh, 0) + ns  (relu fused into the add)
                vt = wpool.tile([P, M], dt.float32, tag="v")
                nc.vector.scalar_tensor_tensor(
                    vt[:rows, :], tt[:rows, :], 0.0, nst[:rows, :],
                    op0=Alu.max, op1=Alu.add,
                )

                mx = opool.tile([P, R], dt.float32, tag="mx")
                ix = opool.tile([P, R], dt.uint32, tag="ix")
                for r in range(3):
                    s = slice(8 * r, 8 * r + 8)
                    nc.vector.max(mx[:rows, s], vt[:rows, :])
                    nc.vector.max_index(ix[:rows, s], mx[:rows, s], vt[:rows, :])
                    if r < 2:
                        nc.vector.match_replace(
                            vt[:rows, :], mx[:rows, s], vt[:rows, :], -1e30
                        )

                nc.sync.dma_start(ov[r0 : r0 + rows, :], mx[:rows, :])
                nc.sync.dma_start(oi[r0 : r0 + rows, :], ix[:rows, :])

    nc.compile()
    return nc


def _get_nc():
    if "nc" not in _compiled:
        _compiled["nc"] = _build()
    return _compiled["nc"]


def _run_device(nodevec, edgevec, ns_s, trace=False):
    from concourse.bass_utils import run_bass_kernel_spmd

    nc = _get_nc()
    nvT = np.ascontiguousarray(nodevec.T.astype(np.float32, copy=False))
    evT = np.ascontiguousarray(edgevec.T.astype(np.float32, copy=False))
    in_maps = [
        {
            "nvT": np.ascontiguousarray(nvT[:, c * RPC : (c + 1) * RPC]),
            "evT": evT,
            "ns": np.ascontiguousarray(ns_s[c * RPC : (c + 1) * RPC]),
        }
        for c in range(NCORES)
    ]
    res = run_bass_kernel_spmd(nc, in_maps, list(range(NCORES)), trace=trace)
    vals = np.concatenate([r["ov"] for r in res.results], axis=0)
    idxs = np.concatenate([r["oi"] for r in res.results], axis=0)
    return vals, idxs, res


def kernel(x, nodevec, edgevec, noise):
    b = x.shape[0]
    n, m = noise.shape
    assert (b, n, m) == (B, N, M), (b, n, m)

    ns_s = (noise.astype(np.float32) * np.float32(0.01)).astype(np.float32)
    vals, idxs, _ = _run_device(nodevec, edgevec, ns_s)

    idx20 = idxs[:, :K].astype(np.int64)
    val20 = vals[:, :K]
    order = np.argsort(idx20, axis=1)
    cols = np.take_along_axis(idx20, order, axis=1)          # [N, K] ascending
    wv = np.take_along_axis(val20, order, axis=1)
    w = (wv - np.take_along_axis(ns_s, cols, axis=1)).astype(np.float32)

    rows = np.broadcast_to(np.arange(N, dtype=np.int64)[:, None], (N, K))
    offb = np.arange(B, dtype=np.int64)
    src = (rows[None, :, :] + (offb * N)[:, None, None]).reshape(-1)
    dst = (cols[None, :, :] + (offb * M)[:, None, None]).reshape(-1)
    HE = np.stack([src, dst], axis=0).astype(np.int32)
    HEW = np.broadcast_to(w.reshape(-1)[None, :], (B, N * K)).reshape(-1)
    HEW = np.ascontiguousarray(HEW, dtype=np.float32)
    return HE, HEW
